# revision 9
# baseline (speedup 1.0000x reference)
"""Two-layer GAT (GraphAttention) forward on 8 Trainium2 NeuronCores.

Math (per layer, reference semantics):
    h  = x @ w                      [N, U]
    a1 = h @ aw1 ; a2 = h @ aw2     [N, H]
    P[i,j,h]    = exp(adj[i,j] * relu(a1[i,h] + a2[j,h]))
    attn[i,j,h] = P / sum_j P
    out[i,h,:]  = sum_j attn[i,j,h] * h[j,:]   -> concat heads -> activation

Key identity used here: with adj in {0,1},
    P[i,j] = max(adj[i,j] * e1[i] * e2[j], 1),   e1 = exp(a1), e2 = exp(a2)
and softmax rows are invariant to scaling by em1[i] = exp(-a1[i]):
    P'[j,i] = max(adjT[j,i] * e2[j], em1[i])
so the per-element work is one tensor_scalar (4x DVE mode, per-partition e2)
plus one tensor_tensor max (2x DVE mode) against a broadcast em1 row, with
numerator and denominator both coming out of a single PE matmul against
[h | 1] extended features.

Sharding: rows (i) of the score matrix are partitioned 512 per core;
adj rows are fed pre-transposed per core as [N, 512] (j on partitions).
All small weights are replicated; hfeat of layer 1 is all-gathered.
"""

import sys

for _p in ("/opt/trn_rl_repo",):
    if _p not in sys.path:
        sys.path.insert(0, _p)

from contextlib import ExitStack

import ml_dtypes
import numpy as np

import concourse.bacc as bacc
import concourse.mybir as mybir
import concourse.tile as tile
from concourse.bass_utils import run_bass_kernel_spmd

F32 = mybir.dt.float32
BF16 = mybir.dt.bfloat16
BF = ml_dtypes.bfloat16

N = 4096          # nodes
FIN = 128         # input features
U0 = 16           # layer-0 units
H0 = 4            # layer-0 heads
NCORES = 8
R = N // NCORES   # local rows per core (512)
NJT = N // 128    # j tiles (32)
GRP = 8           # j-tiles fused per tensor_tensor max
NGRP = NJT // GRP

# Of each group of 8 B-tiles (B = adjT * e2[j]), how many go to DVE
# (tensor_scalar, 4x mode) vs ACT (activation Copy with per-partition scale).
DVE_B_PER_GRP = 3

_CACHE = {}
DEBUG = False


def _build():
    nc = bacc.Bacc("TRN2", target_bir_lowering=False, debug=False,
                   num_devices=NCORES)

    # ---- I/O ----
    d_adjT = nc.dram_tensor("adjT", [N, R], BF16, kind="ExternalInput")
    d_xT = nc.dram_tensor("xT", [FIN, N], F32, kind="ExternalInput")
    d_xTl = nc.dram_tensor("xTl", [FIN, R], F32, kind="ExternalInput")
    d_w0 = nc.dram_tensor("w0", [FIN, U0], F32, kind="ExternalInput")
    d_w0T = nc.dram_tensor("w0T", [U0, FIN], F32, kind="ExternalInput")
    d_aw12 = nc.dram_tensor("aw12", [U0, 2 * H0], F32, kind="ExternalInput")
    d_w1 = nc.dram_tensor("w1", [U0 * H0, 1], F32, kind="ExternalInput")
    d_aw11 = nc.dram_tensor("aw11", [1, 1], F32, kind="ExternalInput")
    d_aw21 = nc.dram_tensor("aw21", [1, 1], F32, kind="ExternalInput")
    d_y = nc.dram_tensor("y", [1, R], F32, kind="ExternalOutput")
    dbg = {}
    if DEBUG:
        for nm, shp, dt in [("em1", [H0, R], BF16), ("e2j", [128, NJT * H0], F32),
                            ("hj", [128, NJT * (U0 + 1)], BF16),
                            ("den", [H0, R], F32), ("h1T", [U0 * H0, R], F32),
                            ("hfeT", [1, R], F32), ("hfe1", [128, NJT], F32),
                            ("em1bc0", [128, R], BF16), ("acc0", [U0 + 1, R], F32),
                            ("e2j1", [128, NJT], F32), ("em1bc1", [128, R], BF16),
                            ("l1acc", [2, R], F32),
                            ("h1raw", [U0 * H0, R], F32), ("recbc", [U0 * H0, R], F32)]:
            dbg[nm] = nc.dram_tensor("dbg_" + nm, shp, dt, kind="ExternalOutput")

    with ExitStack() as ctx:
        tc = ctx.enter_context(tile.TileContext(nc))
        const = ctx.enter_context(tc.tile_pool(name="const", bufs=1))
        work = ctx.enter_context(tc.tile_pool(name="work", bufs=1))
        bpool = ctx.enter_context(tc.tile_pool(name="bpool", bufs=3))
        ppool = ctx.enter_context(tc.tile_pool(name="ppool", bufs=3))
        accs = ctx.enter_context(tc.tile_pool(name="accs", bufs=2))
        dram = ctx.enter_context(tc.tile_pool(name="dram", bufs=1, space="DRAM"))
        pp_misc = ctx.enter_context(tc.tile_pool(name="pp_misc", bufs=2, space="PSUM"))
        pp_hj = ctx.enter_context(tc.tile_pool(name="pp_hj", bufs=2, space="PSUM"))
        pp_acc = ctx.enter_context(tc.tile_pool(name="pp_acc", bufs=3, space="PSUM"))

        # ---- persistent SBUF ----
        sb_adjT = const.tile([128, NJT * R], BF16, tag="adjT")     # 32KB/p
        sb_xT = const.tile([FIN, N], F32, tag="xT")                # 16KB/p
        sb_xTl = const.tile([FIN, R], F32, tag="xTl")
        sb_w0T = const.tile([U0, FIN], F32, tag="w0T")
        sb_aw12 = const.tile([U0, 2 * H0], F32, tag="aw12")
        sb_prep = const.tile([FIN, U0 + H0], F32, tag="prep")      # [w0 | v2]
        sb_v1 = const.tile([FIN, H0], F32, tag="v1")
        sb_w1 = const.tile([U0 * H0, 1], F32, tag="w1")
        sb_aw11 = const.tile([1, 1], F32, tag="aw11")
        sb_naw11 = const.tile([1, 1], F32, tag="naw11")
        sb_aw21bc = const.tile([128, 1], F32, tag="aw21bc")
        sb_hj = const.tile([128, NJT * (U0 + 1)], BF16, tag="hj")  # [h | 1] per jt
        sb_e2j = const.tile([128, NJT * H0], F32, tag="e2j")
        sb_em1 = const.tile([H0, R], BF16, tag="em1")
        sb_em1bc = [const.tile([128, R], BF16, tag=f"em1bc{h}",
                       name=f"em1bc{h}") for h in range(H0)]
        sb_den = const.tile([H0, R], F32, tag="den")
        sb_rec4 = const.tile([H0, R], F32, tag="rec4")
        sb_recbc = const.tile([U0 * H0, R], F32, tag="recbc")
        sb_h1raw = const.tile([U0 * H0, R], F32, tag="h1raw")
        sb_h1T = const.tile([U0 * H0, R], F32, tag="h1T")
        sb_hfeT = const.tile([1, R], F32, tag="hfeT")
        sb_em11 = const.tile([1, R], BF16, tag="em11")
        sb_em1bc1 = const.tile([128, R], BF16, tag="em1bc1")
        sb_hfe1 = const.tile([128, NJT], F32, tag="hfe1")
        sb_e2j1 = const.tile([128, NJT], F32, tag="e2j1")
        sb_hfe1e = const.tile([128, NJT * 2], BF16, tag="hfe1e")
        sb_fin = work.tile([1, R], F32, tag="fin")
        sb_fin2 = work.tile([1, R], F32, tag="fin2")

        d_em1 = dram.tile([H0, R], BF16)
        d_em11 = dram.tile([1, R], BF16)
        d_rec4 = dram.tile([H0, R], F32)
        d_gin = dram.tile([1, R], F32)
        d_gout = dram.tile([NCORES, R], F32)

        # ---- load constants / inputs ----
        nc.sync.dma_start(sb_xT[:], d_xT[:])
        nc.sync.dma_start(sb_xTl[:], d_xTl[:])
        nc.sync.dma_start(sb_w0T[:], d_w0T[:])
        nc.sync.dma_start(sb_aw12[:], d_aw12[:])
        nc.sync.dma_start(sb_prep[:, 0:U0], d_w0[:])
        nc.sync.dma_start(sb_w1[:], d_w1[:])
        nc.sync.dma_start(sb_aw11[:], d_aw11[:])
        nc.sync.dma_start(sb_aw21bc[:], d_aw21[0:1, 0:1].to_broadcast((128, 1)))
        # adjT: 2 j-tiles per DMA for 2KB per-partition lines
        for m in range(NJT // 2):
            src = d_adjT[256 * m:256 * (m + 1), :].rearrange(
                "(g p) i -> p g i", p=128)
            dst = sb_adjT[:, 1024 * m:1024 * (m + 1)].rearrange(
                "p (g i) -> p g i", g=2)
            nc.sync.dma_start(dst, src)

        # ---- prep: v1/v2, a1 -> em1, h/e2 per j-tile ----
        ps_v = pp_misc.tile([FIN, 2 * H0], F32, tag="misc")
        nc.tensor.matmul(ps_v[:], sb_w0T[:], sb_aw12[:], start=True, stop=True)
        nc.scalar.copy(sb_v1[:], ps_v[:, 0:H0])
        nc.scalar.copy(sb_prep[:, U0:U0 + H0], ps_v[:, H0:2 * H0])

        ps_a1 = pp_misc.tile([H0, R], F32, tag="misc")
        nc.tensor.matmul(ps_a1[:], sb_v1[:], sb_xTl[:], start=True, stop=True)
        nc.scalar.activation(sb_em1[:], ps_a1[:],
                             mybir.ActivationFunctionType.Exp, scale=-1.0)
        nc.sync.dma_start(d_em1[:], sb_em1[:])
        for h in range(H0):
            nc.sync.dma_start(sb_em1bc[h][:],
                              d_em1[h:h + 1, :].to_broadcast((128, R)))

        nc.vector.memset(sb_hj[:], 1.0)
        for jt in range(NJT):
            ps_hj = pp_hj.tile([128, U0 + H0], F32, tag="hj")
            nc.tensor.matmul(ps_hj[:], sb_xT[:, 128 * jt:128 * (jt + 1)],
                             sb_prep[:], start=True, stop=True)
            nc.vector.tensor_copy(sb_hj[:, (U0 + 1) * jt:(U0 + 1) * jt + U0],
                                  ps_hj[:, 0:U0])
            nc.scalar.activation(sb_e2j[:, H0 * jt:H0 * (jt + 1)],
                                 ps_hj[:, U0:U0 + H0],
                                 mybir.ActivationFunctionType.Exp)

        # ---- layer 0 main ----
        for h in range(H0):
            ps_acc = pp_acc.tile([U0 + 1, R], F32, tag="acc")
            for g in range(NGRP):
                t_B = bpool.tile([128, GRP * R], BF16, tag="B")
                for k in range(GRP):
                    jt = GRP * g + k
                    dst = t_B[:, R * k:R * (k + 1)]
                    src = sb_adjT[:, R * jt:R * (jt + 1)]
                    sc = sb_e2j[:, H0 * jt + h:H0 * jt + h + 1]
                    if k < DVE_B_PER_GRP:
                        nc.vector.tensor_scalar_mul(dst, src, sc)
                    else:
                        nc.scalar.mul(dst, src, sc)
                t_P = ppool.tile([128, GRP * R], BF16, tag="P")
                nc.vector.tensor_tensor(
                    t_P[:].rearrange("p (g i) -> p g i", g=GRP),
                    t_B[:].rearrange("p (g i) -> p g i", g=GRP),
                    sb_em1bc[h][:, None, :].to_broadcast((128, GRP, R)),
                    mybir.AluOpType.max)
                for k in range(GRP):
                    jt = GRP * g + k
                    nc.tensor.matmul(
                        ps_acc[:],
                        sb_hj[:, (U0 + 1) * jt:(U0 + 1) * (jt + 1)],
                        t_P[:, R * k:R * (k + 1)],
                        start=(jt == 0), stop=(jt == NJT - 1))
            sb_acc = accs.tile([U0 + 1, R], F32, tag="accsb", name="sb_acc")
            nc.vector.tensor_copy(sb_acc[:], ps_acc[:])
            sb_relu = accs.tile([U0, R], F32, tag="relusb", name="sb_relu")
            nc.scalar.activation(sb_relu[:], sb_acc[0:U0, :],
                                 mybir.ActivationFunctionType.Relu)
            nc.sync.dma_start(sb_h1raw[U0 * h:U0 * (h + 1), :], sb_relu[:])
            nc.sync.dma_start(sb_den[h:h + 1, :], sb_acc[U0:U0 + 1, :])
            if DEBUG and h == 0:
                nc.sync.dma_start(dbg["acc0"][:], sb_acc[:])

        nc.vector.reciprocal(sb_rec4[:], sb_den[:])
        nc.sync.dma_start(d_rec4[:], sb_rec4[:])
        for h in range(H0):
            nc.sync.dma_start(sb_recbc[U0 * h:U0 * (h + 1), :],
                              d_rec4[h:h + 1, :].to_broadcast((U0, R)))
        nc.vector.tensor_mul(sb_h1T[:], sb_h1raw[:], sb_recbc[:])

        # ---- layer 1 prep ----
        ps_f = pp_misc.tile([1, R], F32, tag="misc")
        nc.tensor.matmul(ps_f[:], sb_w1[:], sb_h1T[:], start=True, stop=True)
        nc.scalar.mul(sb_naw11[:], sb_aw11[:], -1.0)
        nc.vector.tensor_copy(sb_hfeT[:], ps_f[:])
        nc.scalar.activation(sb_em11[:], ps_f[:],
                             mybir.ActivationFunctionType.Exp,
                             scale=sb_naw11[:])
        nc.sync.dma_start(d_em11[:], sb_em11[:])
        nc.sync.dma_start(sb_em1bc1[:], d_em11[0:1, :].to_broadcast((128, R)))
        nc.sync.dma_start(d_gin[:], sb_hfeT[:])
        nc.gpsimd.collective_compute(
            "AllGather", mybir.AluOpType.bypass,
            replica_groups=[list(range(NCORES))],
            ins=[d_gin[:].opt()], outs=[d_gout[:].opt()])
        gflat = d_gout[:].rearrange("a b -> (a b)").rearrange(
            "(t p) -> p t", p=128)
        nc.sync.dma_start(sb_hfe1[:], gflat)
        nc.scalar.activation(sb_e2j1[:], sb_hfe1[:],
                             mybir.ActivationFunctionType.Exp,
                             scale=sb_aw21bc[:])
        nc.vector.memset(sb_hfe1e[:], 1.0)
        nc.vector.tensor_copy(
            sb_hfe1e[:].rearrange("p (t two) -> p t two", two=2)[:, :, 0:1],
            sb_hfe1[:][:, :, None])

        # ---- layer 1 main ----
        ps_l1 = pp_acc.tile([2, R], F32, tag="acc")
        for g in range(NGRP):
            t_B = bpool.tile([128, GRP * R], BF16, tag="B")
            for k in range(GRP):
                jt = GRP * g + k
                dst = t_B[:, R * k:R * (k + 1)]
                src = sb_adjT[:, R * jt:R * (jt + 1)]
                sc = sb_e2j1[:, jt:jt + 1]
                if k < DVE_B_PER_GRP:
                    nc.vector.tensor_scalar_mul(dst, src, sc)
                else:
                    nc.scalar.mul(dst, src, sc)
            t_P = ppool.tile([128, GRP * R], BF16, tag="P")
            nc.vector.tensor_tensor(
                t_P[:].rearrange("p (g i) -> p g i", g=GRP),
                t_B[:].rearrange("p (g i) -> p g i", g=GRP),
                sb_em1bc1[:, None, :].to_broadcast((128, GRP, R)),
                mybir.AluOpType.max)
            for k in range(GRP):
                jt = GRP * g + k
                nc.tensor.matmul(
                    ps_l1[:], sb_hfe1e[:, 2 * jt:2 * (jt + 1)],
                    t_P[:, R * k:R * (k + 1)],
                    start=(jt == 0), stop=(jt == NJT - 1))

        # ---- final: sigmoid(numer/denom) via 1/(1+exp(-x)) ----
        sb_l1acc = accs.tile([2, R], F32, tag="l1accsb", name="sb_l1acc")
        nc.vector.tensor_copy(sb_l1acc[:], ps_l1[:])
        sb_l1den = accs.tile([1, R], F32, tag="l1densb", name="sb_l1den")
        nc.sync.dma_start(sb_l1den[:], sb_l1acc[1:2, :])
        nc.vector.reciprocal(sb_fin[:], sb_l1den[:])
        nc.vector.tensor_mul(sb_fin2[:], sb_l1acc[0:1, :], sb_fin[:])
        nc.scalar.activation(sb_fin[:], sb_fin2[:],
                             mybir.ActivationFunctionType.Exp, scale=-1.0)
        nc.vector.tensor_scalar_add(sb_fin2[:], sb_fin[:], 1.0)
        nc.vector.reciprocal(sb_fin[:], sb_fin2[:])
        nc.sync.dma_start(d_y[:], sb_fin[:])
        if DEBUG:
            nc.sync.dma_start(dbg["em1"][:], sb_em1[:])
            nc.sync.dma_start(dbg["e2j"][:], sb_e2j[:])
            nc.sync.dma_start(dbg["hj"][:], sb_hj[:])
            nc.sync.dma_start(dbg["den"][:], sb_den[:])
            nc.sync.dma_start(dbg["h1T"][:], sb_h1T[:])
            nc.sync.dma_start(dbg["hfeT"][:], sb_hfeT[:])
            nc.sync.dma_start(dbg["hfe1"][:], sb_hfe1[:])
            nc.sync.dma_start(dbg["em1bc0"][:], sb_em1bc[0][:])
            nc.sync.dma_start(dbg["e2j1"][:], sb_e2j1[:])
            nc.sync.dma_start(dbg["em1bc1"][:], sb_em1bc1[:])
            nc.sync.dma_start(dbg["l1acc"][:], sb_l1acc[:])
            nc.sync.dma_start(dbg["h1raw"][:], sb_h1raw[:])
            nc.sync.dma_start(dbg["recbc"][:], sb_recbc[:])

    nc.compile()
    return nc


def _prep_inputs(x, adj, w0, aw1_0, aw2_0, w1, aw1_1, aw2_1):
    x = np.asarray(x, np.float32)
    adj = np.asarray(adj, np.float32)
    xT = np.ascontiguousarray(x.T)
    adjT = np.asarray(adj.T, BF)                        # [N, N], exact 0/1
    w0 = np.ascontiguousarray(np.asarray(w0, np.float32))
    w0T = np.ascontiguousarray(w0.T)
    aw12 = np.ascontiguousarray(
        np.concatenate([np.asarray(aw1_0, np.float32),
                        np.asarray(aw2_0, np.float32)], axis=1))
    w1 = np.ascontiguousarray(np.asarray(w1, np.float32).reshape(U0 * H0, 1))
    aw11 = np.asarray(aw1_1, np.float32).reshape(1, 1)
    aw21 = np.asarray(aw2_1, np.float32).reshape(1, 1)
    in_maps = []
    for c in range(NCORES):
        rows = slice(R * c, R * (c + 1))
        in_maps.append({
            "adjT": np.ascontiguousarray(adjT[:, rows]),
            "xT": xT,
            "xTl": np.ascontiguousarray(xT[:, rows]),
            "w0": w0, "w0T": w0T, "aw12": aw12, "w1": w1,
            "aw11": aw11, "aw21": aw21,
        })
    return in_maps


def run(inputs, trace=False):
    if "nc" not in _CACHE:
        _CACHE["nc"] = _build()
    nc = _CACHE["nc"]
    in_maps = _prep_inputs(**inputs)
    res = run_bass_kernel_spmd(nc, in_maps, list(range(NCORES)), trace=trace)
    y = np.concatenate([res.results[c]["y"][0] for c in range(NCORES)])
    return np.ascontiguousarray(y.astype(np.float32)), res


def kernel(**inputs):
    y, _ = run(inputs)
    return y


# revision 10
# speedup vs baseline: 1.0099x; 1.0099x over previous
"""Two-layer GAT (GraphAttention) forward on 8 Trainium2 NeuronCores.

Math (per layer, reference semantics):
    h  = x @ w                      [N, U]
    a1 = h @ aw1 ; a2 = h @ aw2     [N, H]
    P[i,j,h]    = exp(adj[i,j] * relu(a1[i,h] + a2[j,h]))
    attn[i,j,h] = P / sum_j P
    out[i,h,:]  = sum_j attn[i,j,h] * h[j,:]   -> concat heads -> activation

Key identity used here: with adj in {0,1},
    P[i,j] = max(adj[i,j] * e1[i] * e2[j], 1),   e1 = exp(a1), e2 = exp(a2)
and softmax rows are invariant to scaling by em1[i] = exp(-a1[i]):
    P'[j,i] = max(adjT[j,i] * e2[j], em1[i])
so the per-element work is one tensor_scalar (4x DVE mode, per-partition e2)
plus one tensor_tensor max (2x DVE mode) against a broadcast em1 row, with
numerator and denominator both coming out of a single PE matmul against
[h | 1] extended features.

Sharding: rows (i) of the score matrix are partitioned 512 per core;
adj rows are fed pre-transposed per core as [N, 512] (j on partitions).
All small weights are replicated; hfeat of layer 1 is all-gathered.
"""

import sys

for _p in ("/opt/trn_rl_repo",):
    if _p not in sys.path:
        sys.path.insert(0, _p)

from contextlib import ExitStack

import ml_dtypes
import numpy as np

import concourse.bacc as bacc
import concourse.mybir as mybir
import concourse.tile as tile
from concourse.bass_utils import run_bass_kernel_spmd

F32 = mybir.dt.float32
BF16 = mybir.dt.bfloat16
BF = ml_dtypes.bfloat16

N = 4096          # nodes
FIN = 128         # input features
U0 = 16           # layer-0 units
H0 = 4            # layer-0 heads
NCORES = 8
R = N // NCORES   # local rows per core (512)
NJT = N // 128    # j tiles (32)
GRP = 8           # j-tiles fused per tensor_tensor max
NGRP = NJT // GRP

# Of each group of 8 B-tiles (B = adjT * e2[j]), how many go to DVE
# (tensor_scalar, 4x mode) vs ACT (activation Copy with per-partition scale).
DVE_B_PER_GRP = 4

_CACHE = {}
DEBUG = False


def _build():
    nc = bacc.Bacc("TRN2", target_bir_lowering=False, debug=False,
                   num_devices=NCORES)

    # ---- I/O ----
    d_adjT = nc.dram_tensor("adjT", [N, R], BF16, kind="ExternalInput")
    d_xT = nc.dram_tensor("xT", [FIN, N], BF16, kind="ExternalInput")
    d_xTl = nc.dram_tensor("xTl", [FIN, R], BF16, kind="ExternalInput")
    d_w0 = nc.dram_tensor("w0", [FIN, U0], BF16, kind="ExternalInput")
    d_w0T = nc.dram_tensor("w0T", [U0, FIN], F32, kind="ExternalInput")
    d_aw12 = nc.dram_tensor("aw12", [U0, 2 * H0], F32, kind="ExternalInput")
    d_w1 = nc.dram_tensor("w1", [U0 * H0, 1], F32, kind="ExternalInput")
    d_aw11 = nc.dram_tensor("aw11", [1, 1], F32, kind="ExternalInput")
    d_aw21 = nc.dram_tensor("aw21", [1, 1], F32, kind="ExternalInput")
    d_y = nc.dram_tensor("y", [1, R], F32, kind="ExternalOutput")
    dbg = {}
    if DEBUG:
        for nm, shp, dt in [("em1", [H0, R], BF16), ("e2j", [128, NJT * H0], F32),
                            ("hj", [128, NJT * (U0 + 1)], BF16),
                            ("den", [H0, R], F32), ("h1T", [U0 * H0, R], F32),
                            ("hfeT", [1, R], F32), ("hfe1", [128, NJT], F32),
                            ("em1bc0", [128, R], BF16), ("acc0", [U0 + 1, R], F32),
                            ("e2j1", [128, NJT], F32), ("em1bc1", [128, R], BF16),
                            ("l1acc", [2, R], F32),
                            ("h1raw", [U0 * H0, R], F32), ("recbc", [U0 * H0, R], F32)]:
            dbg[nm] = nc.dram_tensor("dbg_" + nm, shp, dt, kind="ExternalOutput")

    with ExitStack() as ctx:
        tc = ctx.enter_context(tile.TileContext(nc))
        const = ctx.enter_context(tc.tile_pool(name="const", bufs=1))
        work = ctx.enter_context(tc.tile_pool(name="work", bufs=1))
        bpool = ctx.enter_context(tc.tile_pool(name="bpool", bufs=3))
        ppool = ctx.enter_context(tc.tile_pool(name="ppool", bufs=3))
        accs = ctx.enter_context(tc.tile_pool(name="accs", bufs=2))
        dram = ctx.enter_context(tc.tile_pool(name="dram", bufs=1, space="DRAM"))
        pp_misc = ctx.enter_context(tc.tile_pool(name="pp_misc", bufs=2, space="PSUM"))
        pp_hj = ctx.enter_context(tc.tile_pool(name="pp_hj", bufs=2, space="PSUM"))
        pp_acc = ctx.enter_context(tc.tile_pool(name="pp_acc", bufs=3, space="PSUM"))

        # ---- persistent SBUF ----
        sb_adjT = const.tile([128, NJT * R], BF16, tag="adjT")     # 32KB/p
        sb_xT = const.tile([FIN, N], BF16, tag="xT")               # 8KB/p
        sb_xTl = const.tile([FIN, R], BF16, tag="xTl")
        sb_w0T = const.tile([U0, FIN], F32, tag="w0T")
        sb_aw12 = const.tile([U0, 2 * H0], F32, tag="aw12")
        sb_prep = const.tile([FIN, U0 + H0], BF16, tag="prep")     # [w0 | v2]
        sb_v1 = const.tile([FIN, H0], BF16, tag="v1")
        sb_w1 = const.tile([U0 * H0, 1], F32, tag="w1")
        sb_aw11 = const.tile([1, 1], F32, tag="aw11")
        sb_naw11 = const.tile([1, 1], F32, tag="naw11")
        sb_aw21bc = const.tile([128, 1], F32, tag="aw21bc")
        sb_hj = const.tile([128, NJT * (U0 + 1)], BF16, tag="hj")  # [h | 1] per jt
        sb_e2j = const.tile([128, NJT * H0], F32, tag="e2j")
        sb_em1 = const.tile([H0, R], BF16, tag="em1")
        sb_em1bc = [const.tile([128, R], BF16, tag=f"em1bc{h}",
                       name=f"em1bc{h}") for h in range(H0)]
        sb_den = const.tile([H0, R], F32, tag="den")
        sb_rec4 = const.tile([H0, R], F32, tag="rec4")
        sb_recbc = const.tile([U0 * H0, R], F32, tag="recbc")
        sb_h1raw = const.tile([U0 * H0, R], F32, tag="h1raw")
        sb_h1T = const.tile([U0 * H0, R], F32, tag="h1T")
        sb_hfeT = const.tile([1, R], F32, tag="hfeT")
        sb_em11 = const.tile([1, R], BF16, tag="em11")
        sb_em1bc1 = const.tile([128, R], BF16, tag="em1bc1")
        sb_hfe1 = const.tile([128, NJT], F32, tag="hfe1")
        sb_e2j1 = const.tile([128, NJT], F32, tag="e2j1")
        sb_hfe1e = const.tile([128, NJT * 2], BF16, tag="hfe1e")
        sb_fin = work.tile([1, R], F32, tag="fin")
        sb_fin2 = work.tile([1, R], F32, tag="fin2")

        d_em1 = dram.tile([H0, R], BF16)
        d_em11 = dram.tile([1, R], BF16)
        d_rec4 = dram.tile([H0, R], F32)
        d_gin = dram.tile([1, R], F32)
        d_gout = dram.tile([NCORES, R], F32)

        # ---- load constants / inputs ----
        nc.sync.dma_start(sb_xT[:], d_xT[:])
        nc.sync.dma_start(sb_xTl[:], d_xTl[:])
        nc.sync.dma_start(sb_w0T[:], d_w0T[:])
        nc.sync.dma_start(sb_aw12[:], d_aw12[:])
        nc.sync.dma_start(sb_prep[:, 0:U0], d_w0[:])
        nc.sync.dma_start(sb_w1[:], d_w1[:])
        nc.sync.dma_start(sb_aw11[:], d_aw11[:])
        nc.sync.dma_start(sb_aw21bc[:], d_aw21[0:1, 0:1].to_broadcast((128, 1)))
        # adjT: 2 j-tiles per DMA for 2KB per-partition lines
        for m in range(NJT // 2):
            src = d_adjT[256 * m:256 * (m + 1), :].rearrange(
                "(g p) i -> p g i", p=128)
            dst = sb_adjT[:, 1024 * m:1024 * (m + 1)].rearrange(
                "p (g i) -> p g i", g=2)
            nc.sync.dma_start(dst, src)

        # ---- prep: v1/v2, a1 -> em1, h/e2 per j-tile ----
        ps_v = pp_misc.tile([FIN, 2 * H0], F32, tag="misc")
        nc.tensor.matmul(ps_v[:], sb_w0T[:], sb_aw12[:], start=True, stop=True)
        nc.scalar.copy(sb_v1[:], ps_v[:, 0:H0])
        nc.scalar.copy(sb_prep[:, U0:U0 + H0], ps_v[:, H0:2 * H0])

        ps_a1 = pp_misc.tile([H0, R], F32, tag="misc")
        nc.tensor.matmul(ps_a1[:], sb_v1[:], sb_xTl[:], start=True, stop=True)
        nc.scalar.activation(sb_em1[:], ps_a1[:],
                             mybir.ActivationFunctionType.Exp, scale=-1.0)
        nc.sync.dma_start(d_em1[:], sb_em1[:])
        for h in range(H0):
            nc.sync.dma_start(sb_em1bc[h][:],
                              d_em1[h:h + 1, :].to_broadcast((128, R)))

        nc.vector.memset(sb_hj[:], 1.0)
        W = U0 + H0
        for q4 in range(NJT // 4):
            ps4 = pp_hj.tile([128, 4 * W], F32, tag="hj", name="ps4")
            for q in range(4):
                jt = 4 * q4 + q
                nc.tensor.matmul(ps4[:, W * q:W * (q + 1)],
                                 sb_xT[:, 128 * jt:128 * (jt + 1)],
                                 sb_prep[:], start=True, stop=True)
            hjv = sb_hj[:, 4 * (U0 + 1) * q4:4 * (U0 + 1) * (q4 + 1)].rearrange(
                "p (q c) -> p q c", q=4)[:, :, 0:U0]
            psv = ps4[:].rearrange("p (q c) -> p q c", q=4)[:, :, 0:U0]
            nc.vector.tensor_copy(hjv, psv)
            e2v = sb_e2j[:, 4 * H0 * q4:4 * H0 * (q4 + 1)].rearrange(
                "p (q c) -> p q c", q=4)
            pse = ps4[:].rearrange("p (q c) -> p q c", q=4)[:, :, U0:U0 + H0]
            nc.scalar.activation(e2v, pse, mybir.ActivationFunctionType.Exp)

        # ---- layer 0 main ----
        for h in range(H0):
            ps_acc = pp_acc.tile([U0 + 1, R], F32, tag="acc")
            for g in range(NGRP):
                t_B = bpool.tile([128, GRP * R], BF16, tag="B")
                for k in range(GRP):
                    jt = GRP * g + k
                    dst = t_B[:, R * k:R * (k + 1)]
                    src = sb_adjT[:, R * jt:R * (jt + 1)]
                    sc = sb_e2j[:, H0 * jt + h:H0 * jt + h + 1]
                    if k < DVE_B_PER_GRP:
                        nc.vector.tensor_scalar_mul(dst, src, sc)
                    else:
                        nc.scalar.mul(dst, src, sc)
                t_P = ppool.tile([128, GRP * R], BF16, tag="P")
                nc.vector.tensor_tensor(
                    t_P[:].rearrange("p (g i) -> p g i", g=GRP),
                    t_B[:].rearrange("p (g i) -> p g i", g=GRP),
                    sb_em1bc[h][:, None, :].to_broadcast((128, GRP, R)),
                    mybir.AluOpType.max)
                for k in range(GRP):
                    jt = GRP * g + k
                    nc.tensor.matmul(
                        ps_acc[:],
                        sb_hj[:, (U0 + 1) * jt:(U0 + 1) * (jt + 1)],
                        t_P[:, R * k:R * (k + 1)],
                        start=(jt == 0), stop=(jt == NJT - 1))
            sb_acc = accs.tile([U0 + 1, R], F32, tag="accsb", name="sb_acc")
            nc.vector.tensor_copy(sb_acc[:], ps_acc[:])
            sb_relu = accs.tile([U0, R], F32, tag="relusb", name="sb_relu")
            nc.scalar.activation(sb_relu[:], sb_acc[0:U0, :],
                                 mybir.ActivationFunctionType.Relu)
            nc.sync.dma_start(sb_h1raw[U0 * h:U0 * (h + 1), :], sb_relu[:])
            nc.sync.dma_start(sb_den[h:h + 1, :], sb_acc[U0:U0 + 1, :])
            if DEBUG and h == 0:
                nc.sync.dma_start(dbg["acc0"][:], sb_acc[:])

        sb_rscr = accs.tile([H0, R], F32, tag="rscr", name="sb_rscr")
        nc.vector.reciprocal_approx_accurate(sb_rec4[:], sb_den[:], sb_rscr[:])
        nc.sync.dma_start(d_rec4[:], sb_rec4[:])
        for h in range(H0):
            nc.sync.dma_start(sb_recbc[U0 * h:U0 * (h + 1), :],
                              d_rec4[h:h + 1, :].to_broadcast((U0, R)))
        nc.vector.tensor_mul(sb_h1T[:], sb_h1raw[:], sb_recbc[:])

        # ---- layer 1 prep ----
        ps_f = pp_misc.tile([1, R], F32, tag="misc")
        nc.tensor.matmul(ps_f[:], sb_w1[:], sb_h1T[:], start=True, stop=True)
        nc.scalar.mul(sb_naw11[:], sb_aw11[:], -1.0)
        nc.vector.tensor_copy(sb_hfeT[:], ps_f[:])
        nc.scalar.activation(sb_em11[:], ps_f[:],
                             mybir.ActivationFunctionType.Exp,
                             scale=sb_naw11[:])
        nc.sync.dma_start(d_em11[:], sb_em11[:])
        nc.sync.dma_start(sb_em1bc1[:], d_em11[0:1, :].to_broadcast((128, R)))
        nc.sync.dma_start(d_gin[:], sb_hfeT[:])
        nc.gpsimd.collective_compute(
            "AllGather", mybir.AluOpType.bypass,
            replica_groups=[list(range(NCORES))],
            ins=[d_gin[:].opt()], outs=[d_gout[:].opt()])
        gflat = d_gout[:].rearrange("a b -> (a b)").rearrange(
            "(t p) -> p t", p=128)
        nc.sync.dma_start(sb_hfe1[:], gflat)
        nc.scalar.activation(sb_e2j1[:], sb_hfe1[:],
                             mybir.ActivationFunctionType.Exp,
                             scale=sb_aw21bc[:])
        nc.vector.memset(sb_hfe1e[:], 1.0)
        nc.vector.tensor_copy(
            sb_hfe1e[:].rearrange("p (t two) -> p t two", two=2)[:, :, 0:1],
            sb_hfe1[:][:, :, None])

        # ---- layer 1 main ----
        ps_l1 = pp_acc.tile([2, R], F32, tag="acc")
        for g in range(NGRP):
            t_B = bpool.tile([128, GRP * R], BF16, tag="B")
            for k in range(GRP):
                jt = GRP * g + k
                dst = t_B[:, R * k:R * (k + 1)]
                src = sb_adjT[:, R * jt:R * (jt + 1)]
                sc = sb_e2j1[:, jt:jt + 1]
                if k < DVE_B_PER_GRP:
                    nc.vector.tensor_scalar_mul(dst, src, sc)
                else:
                    nc.scalar.mul(dst, src, sc)
            t_P = ppool.tile([128, GRP * R], BF16, tag="P")
            nc.vector.tensor_tensor(
                t_P[:].rearrange("p (g i) -> p g i", g=GRP),
                t_B[:].rearrange("p (g i) -> p g i", g=GRP),
                sb_em1bc1[:, None, :].to_broadcast((128, GRP, R)),
                mybir.AluOpType.max)
            for k in range(GRP):
                jt = GRP * g + k
                nc.tensor.matmul(
                    ps_l1[:], sb_hfe1e[:, 2 * jt:2 * (jt + 1)],
                    t_P[:, R * k:R * (k + 1)],
                    start=(jt == 0), stop=(jt == NJT - 1))

        # ---- final: sigmoid(numer/denom) via 1/(1+exp(-x)) ----
        sb_l1acc = accs.tile([2, R], F32, tag="l1accsb", name="sb_l1acc")
        nc.vector.tensor_copy(sb_l1acc[:], ps_l1[:])
        sb_l1den = accs.tile([1, R], F32, tag="l1densb", name="sb_l1den")
        nc.sync.dma_start(sb_l1den[:], sb_l1acc[1:2, :])
        sb_fscr = accs.tile([1, R], F32, tag="fscr", name="sb_fscr")
        nc.vector.reciprocal_approx_accurate(sb_fin[:], sb_l1den[:], sb_fscr[:])
        nc.vector.tensor_mul(sb_fin2[:], sb_l1acc[0:1, :], sb_fin[:])
        nc.scalar.activation(sb_fin[:], sb_fin2[:],
                             mybir.ActivationFunctionType.Exp, scale=-1.0)
        nc.vector.tensor_scalar_add(sb_fin2[:], sb_fin[:], 1.0)
        nc.vector.reciprocal_approx_accurate(sb_fin[:], sb_fin2[:], sb_fscr[:])
        nc.sync.dma_start(d_y[:], sb_fin[:])
        if DEBUG:
            nc.sync.dma_start(dbg["em1"][:], sb_em1[:])
            nc.sync.dma_start(dbg["e2j"][:], sb_e2j[:])
            nc.sync.dma_start(dbg["hj"][:], sb_hj[:])
            nc.sync.dma_start(dbg["den"][:], sb_den[:])
            nc.sync.dma_start(dbg["h1T"][:], sb_h1T[:])
            nc.sync.dma_start(dbg["hfeT"][:], sb_hfeT[:])
            nc.sync.dma_start(dbg["hfe1"][:], sb_hfe1[:])
            nc.sync.dma_start(dbg["em1bc0"][:], sb_em1bc[0][:])
            nc.sync.dma_start(dbg["e2j1"][:], sb_e2j1[:])
            nc.sync.dma_start(dbg["em1bc1"][:], sb_em1bc1[:])
            nc.sync.dma_start(dbg["l1acc"][:], sb_l1acc[:])
            nc.sync.dma_start(dbg["h1raw"][:], sb_h1raw[:])
            nc.sync.dma_start(dbg["recbc"][:], sb_recbc[:])

    nc.compile()
    return nc


def _prep_inputs(x, adj, w0, aw1_0, aw2_0, w1, aw1_1, aw2_1):
    x = np.asarray(x, np.float32)
    adj = np.asarray(adj, np.float32)
    xT = np.ascontiguousarray(x.T.astype(BF))
    adjT = np.asarray(adj.T, BF)                        # [N, N], exact 0/1
    w0f = np.asarray(w0, np.float32)
    w0 = np.ascontiguousarray(w0f.astype(BF))
    w0T = np.ascontiguousarray(w0f.T)
    aw12 = np.ascontiguousarray(
        np.concatenate([np.asarray(aw1_0, np.float32),
                        np.asarray(aw2_0, np.float32)], axis=1))
    w1 = np.ascontiguousarray(np.asarray(w1, np.float32).reshape(U0 * H0, 1))
    aw11 = np.asarray(aw1_1, np.float32).reshape(1, 1)
    aw21 = np.asarray(aw2_1, np.float32).reshape(1, 1)
    in_maps = []
    for c in range(NCORES):
        rows = slice(R * c, R * (c + 1))
        in_maps.append({
            "adjT": np.ascontiguousarray(adjT[:, rows]),
            "xT": xT,
            "xTl": np.ascontiguousarray(xT[:, rows]),
            "w0": w0, "w0T": w0T, "aw12": aw12, "w1": w1,
            "aw11": aw11, "aw21": aw21,
        })
    return in_maps


def run(inputs, trace=False):
    if "nc" not in _CACHE:
        _CACHE["nc"] = _build()
    nc = _CACHE["nc"]
    in_maps = _prep_inputs(**inputs)
    res = run_bass_kernel_spmd(nc, in_maps, list(range(NCORES)), trace=trace)
    y = np.concatenate([res.results[c]["y"][0] for c in range(NCORES)])
    return np.ascontiguousarray(y.astype(np.float32)), res


def kernel(**inputs):
    y, _ = run(inputs)
    return y
